# revision 5
# baseline (speedup 1.0000x reference)
"""NTK-ViT self-attention (softmax attention + linear-attention correction)
for Trainium2, data-parallel over batch across 8 NeuronCores.

Math (per batch b, head h):
    q = hidden @ Wq.T + bq ; k = .. ; v = ..           (per-head slices of 768)
    A'  = exp(q k^T / sqrt(d))          (no max-subtract: values are O(exp(6)),
                                         and the reference's max-subtract cancels
                                         exactly in the final ratio)
    phi_q = elu(q / d^0.25) + 1 = exp(min(x,0)) + relu(x),  x = q / d^0.25
    ctx = (A' v + phi_q @ phi_kv) / (rowsum(A') + phi_q @ |phi_k|)

On-chip layout strategy (per core: 2 batches, 12 heads = 6 head-pairs):
  - hidden^T, W^T via PE transposes (bf16).
  - Q^T, K^T in [o=768 (6 x 128-part tiles = head PAIRS), s=1024] bf16.
  - scores computed transposed: S^T[t, q] = (K^T)^T-contraction; the two heads
    of a pair run CONCURRENTLY on the PE array (K=64 row tiling via
    base_partition 0 / 64).
  - exp on ACT (scale=1/8 fused), writing bf16 A'^T tiles [t_tile, q].
  - A'v: out[q_tile, 65] psum accumulation with lhsT = A'^T tiles and
    rhs = V_aug = [V | ones]; the ones column yields rowsum(A') for free.
    The phi-correction matmul accumulates into the same psum.
  - epilogue: reciprocal of col 64, scale cols 0..63, DMA straight to DRAM.
"""

import numpy as np

import concourse.bass as bass
import concourse.mybir as mybir
import concourse.tile as tile
import bass_rust
from concourse.vector_clock import ScopedClock
from concourse.masks import make_identity
from concourse.bass_utils import run_bass_kernel_spmd

F32 = mybir.dt.float32
BF16 = mybir.dt.bfloat16
AF = mybir.ActivationFunctionType
ALU = mybir.AluOpType

B, S, HID = 16, 1024, 768
H, DH = 12, 64
NCORES = 8
BLOC = B // NCORES          # batches per core
SLOC = BLOC * S             # 2048 rows of hidden per core
HP = H // 2                 # head pairs
INV_SQRT_D = 1.0 / np.sqrt(DH)          # 0.125
INV_QD = 1.0 / float(DH) ** 0.25        # 1/2.8284


def _split_multi_waits(nc):
    """This walrus build rejects instructions carrying more than one sync
    wait. Hoist extra waits onto standalone EventSemaphore nops emitted
    immediately before the instruction on the same engine (identical
    blocking semantics: the engine stalls on each wait in turn)."""
    ctr = 0
    for fn in nc.m.functions:
        for bb in fn.blocks:
            out = []
            changed = False
            for inst in bb.instructions:
                si = inst.sync_info
                if si is not None and len(si.on_wait) > 1:
                    waits = list(si.on_wait)
                    for w in waits[:-1]:
                        ctr += 1
                        nop = mybir.InstEventSemaphore(
                            name=f"I-waitsplit-{ctr}",
                            engine=inst.engine,
                            ins=[], outs=[],
                            sync_info=bass_rust.SyncInfo(
                                on_wait=[w], on_update=[]),
                        )
                        out.append(nop)
                    inst.sync_info = bass_rust.SyncInfo(
                        on_wait=[waits[-1]], on_update=list(si.on_update))
                    changed = True
                out.append(inst)
            if changed:
                bb.instructions = out


def build_nc():
    nc = bass.Bass()
    hid = nc.declare_dram_parameter("hidden", [SLOC, HID], F32, isOutput=False)
    w_in = {
        w: nc.declare_dram_parameter(w, [HID, HID], F32, isOutput=False)
        for w in ("wq", "wk", "wv")
    }
    bq_p = nc.declare_dram_parameter("bq_p", [128, 6], F32, isOutput=False)
    bk_p = nc.declare_dram_parameter("bk_p", [128, 6], F32, isOutput=False)
    bv_b = nc.declare_dram_parameter("bv_b", [128, HID], F32, isOutput=False)
    # [128 (=2x d), H, DH+1]: [phi_kv | |phi_k|], replicated on both 64-halves
    pkv = nc.declare_dram_parameter("phikv_aug", [128, H, DH + 1], F32,
                                    isOutput=False)
    outp = nc.declare_dram_parameter("out", [SLOC, HID], F32, isOutput=True)

    with tile.TileContext(nc) as tc:
        import contextlib
        with contextlib.ExitStack() as ctx:
            cpool = ctx.enter_context(tc.tile_pool(name="const", bufs=1))
            wtp = ctx.enter_context(tc.tile_pool(name="wt", bufs=1))
            big = ctx.enter_context(tc.tile_pool(name="big", bufs=1))
            att = ctx.enter_context(tc.tile_pool(name="att", bufs=2))
            tmp = ctx.enter_context(tc.tile_pool(name="tmp", bufs=3))
            eps_p = ctx.enter_context(tc.tile_pool(name="eps", bufs=6))
            ptr = ctx.enter_context(tc.tile_pool(name="ptr", bufs=1, space="PSUM"))
            pproj = ctx.enter_context(tc.tile_pool(name="pproj", bufs=1, space="PSUM"))
            pqk = ctx.enter_context(tc.tile_pool(name="pqk", bufs=2, space="PSUM"))
            pav = ctx.enter_context(tc.tile_pool(name="pav", bufs=2, space="PSUM"))

            # ---- constants ----
            ident = cpool.tile([128, 128], BF16)
            make_identity(nc, ident[:])

            pkv_f = cpool.tile([128, H, DH + 1], F32)
            nc.sync.dma_start(out=pkv_f[:], in_=pkv[:])
            pkv_b = cpool.tile([128, H, DH + 1], BF16)
            nc.vector.tensor_copy(out=pkv_b[:], in_=pkv_f[:])

            bqt = cpool.tile([128, 6], F32)
            nc.sync.dma_start(out=bqt[:], in_=bq_p[:])
            bkt = cpool.tile([128, 6], F32)
            nc.sync.dma_start(out=bkt[:], in_=bk_p[:])
            bvt = cpool.tile([128, HID], F32)
            nc.sync.dma_start(out=bvt[:], in_=bv_b[:])

            # ---- W^T (bf16) via PE transposes: WT[w][i_part, i_tile, o] ----
            wt = {}
            for w in ("wq", "wk", "wv"):
                wt[w] = wtp.tile([128, 6, HID], BF16, tag=f"wt_{w}",
                                 name=f"wt_{w}")
                for ot in range(6):
                    row = tmp.tile([128, HID], F32, tag="row_f")
                    nc.sync.dma_start(out=row[:], in_=w_in[w][ot * 128:(ot + 1) * 128, :])
                    rowb = tmp.tile([128, HID], BF16, tag="row_b")
                    nc.vector.tensor_copy(out=rowb[:], in_=row[:])
                    for it in range(6):
                        pt = ptr.tile([128, 128], BF16, tag="tr")
                        nc.tensor.transpose(pt[:], rowb[:, it * 128:(it + 1) * 128],
                                            ident[:])
                        nc.vector.tensor_copy(
                            out=wt[w][:, it, ot * 128:(ot + 1) * 128], in_=pt[:])

            for b in range(BLOC):
                # ---- hidden^T (bf16): hT[i_part, i_tile, s] ----
                hT = big.tile([128, 6, S], BF16, tag="hT")
                for st in range(8):
                    row = tmp.tile([128, HID], F32, tag="row_f")
                    nc.sync.dma_start(
                        out=row[:], in_=hid[b * S + st * 128: b * S + (st + 1) * 128, :])
                    rowb = tmp.tile([128, HID], BF16, tag="row_b")
                    nc.vector.tensor_copy(out=rowb[:], in_=row[:])
                    for it in range(6):
                        pt = ptr.tile([128, 128], BF16, tag="tr")
                        nc.tensor.transpose(pt[:], rowb[:, it * 128:(it + 1) * 128],
                                            ident[:])
                        nc.vector.tensor_copy(
                            out=hT[:, it, st * 128:(st + 1) * 128], in_=pt[:])

                # ---- Q^T, K^T projections: QT[o_part, hp, s] (bf16, biased) ----
                QT = big.tile([128, HP, S], BF16, tag="QT")
                KT = big.tile([128, HP, S], BF16, tag="KT")
                for dst, wkey, bias in ((QT, "wq", bqt), (KT, "wk", bkt)):
                    for hp in range(HP):
                        for qc in range(2):
                            ps = pproj.tile([128, 512], F32, tag="proj")
                            for it in range(6):
                                nc.tensor.matmul(
                                    ps[:],
                                    lhsT=wt[wkey][:, it, hp * 128:(hp + 1) * 128],
                                    rhs=hT[:, it, qc * 512:(qc + 1) * 512],
                                    start=(it == 0), stop=(it == 5))
                            nc.vector.tensor_scalar(
                                out=dst[:, hp, qc * 512:(qc + 1) * 512],
                                in0=ps[:], scalar1=bias[:, hp:hp + 1], scalar2=None,
                                op0=ALU.add)

                # ---- V (natural [t, o]) + ones column: V_aug[t_part, t_tile, h, 65]
                vaug = big.tile([128, 8, H, DH + 1], BF16, tag="vaug")
                nc.any.memset(vaug[:, :, :, DH:DH + 1], 1.0)
                for st in range(8):
                    for nc2 in range(2):
                        ps = pproj.tile([128, 512], F32, tag="proj")
                        for it in range(6):
                            nc.tensor.matmul(
                                ps[:, :384],
                                lhsT=hT[:, it, st * 128:(st + 1) * 128],
                                rhs=wt["wv"][:, it, nc2 * 384:(nc2 + 1) * 384],
                                start=(it == 0), stop=(it == 5))
                        nc.vector.tensor_tensor(
                            out=vaug[:, st, nc2 * 6:(nc2 + 1) * 6, 0:DH],
                            in0=ps[:, :384].rearrange("p (h d) -> p h d", d=DH),
                            in1=bvt[:, nc2 * 384:(nc2 + 1) * 384].rearrange(
                                "p (h d) -> p h d", d=DH),
                            op=ALU.add)

                # ---- phi_q^T = elu(q/d^.25)+1 = exp(min(x,0)) + relu(x) ----
                phiQ = big.tile([128, HP, S], BF16, tag="phiQ")
                for hp in range(HP):
                    t1 = tmp.tile([128, S], F32, tag="phi_min")
                    nc.vector.tensor_scalar_min(t1[:], QT[:, hp, :], 0.0)
                    t2 = tmp.tile([128, S], BF16, tag="phi_exp")
                    nc.scalar.activation(t2[:], t1[:], AF.Exp, scale=INV_QD)
                    nc.vector.tensor_scalar(
                        out=phiQ[:, hp, :], in0=QT[:, hp, :],
                        scalar1=0.0, scalar2=INV_QD, op0=ALU.max, op1=ALU.mult)
                    nc.vector.tensor_tensor(
                        out=phiQ[:, hp, :], in0=phiQ[:, hp, :], in1=t2[:],
                        op=ALU.add)

                # ---- attention per head pair ----
                for hp in range(HP):
                    AT = att.tile([128, 2, 8, S], BF16, tag="AT")
                    for t in range(8):
                        for h2 in range(2):
                            lo, hi = h2 * 64, (h2 + 1) * 64
                            ps = pqk.tile([128, 1024], F32, tag="qk")
                            for qc in range(2):
                                nc.tensor.matmul(
                                    ps[:, qc * 512:(qc + 1) * 512],
                                    lhsT=KT[lo:hi, hp, t * 128:(t + 1) * 128],
                                    rhs=QT[lo:hi, hp, qc * 512:(qc + 1) * 512],
                                    start=True, stop=True)
                            nc.scalar.activation(AT[:, h2, t, :], ps[:], AF.Exp,
                                                 scale=INV_SQRT_D)
                    for h2 in range(2):
                        h_abs = hp * 2 + h2
                        lo, hi = h2 * 64, (h2 + 1) * 64
                        for qt in range(8):
                            po = pav.tile([128, DH + 1], F32, tag="av")
                            for t in range(8):
                                nc.tensor.matmul(
                                    po[:],
                                    lhsT=AT[:, h2, t, qt * 128:(qt + 1) * 128],
                                    rhs=vaug[:, t, h_abs, :],
                                    start=(t == 0), stop=False,
                                    skip_group_check=True)
                            nc.tensor.matmul(
                                po[:],
                                lhsT=phiQ[lo:hi, hp, qt * 128:(qt + 1) * 128],
                                rhs=pkv_b[lo:hi, h_abs, :],
                                start=False, stop=True, skip_group_check=True)
                            r = eps_p.tile([128, 1], F32, tag="recip")
                            nc.vector.reciprocal(r[:], po[:, DH:DH + 1])
                            stg = eps_p.tile([128, DH], F32, tag="stage")
                            nc.vector.tensor_scalar(
                                out=stg[:], in0=po[:, 0:DH], scalar1=r[:],
                                scalar2=None, op0=ALU.mult)
                            nc.sync.dma_start(
                                out=outp[b * S + qt * 128: b * S + (qt + 1) * 128,
                                         h_abs * DH:(h_abs + 1) * DH],
                                in_=stg[:])
    _split_multi_waits(nc)
    return nc


_CACHE = {}


def _prep_in_maps(hidden_states, Wq, bq, Wk, bk, Wv, bv, phi_k, phi_kv):
    hidden = np.ascontiguousarray(
        np.asarray(hidden_states, np.float32).reshape(B, S, HID))
    wq = np.ascontiguousarray(np.asarray(Wq, np.float32))
    wk = np.ascontiguousarray(np.asarray(Wk, np.float32))
    wv = np.ascontiguousarray(np.asarray(Wv, np.float32))
    bq_p = np.ascontiguousarray(np.asarray(bq, np.float32).reshape(6, 128).T)
    bk_p = np.ascontiguousarray(np.asarray(bk, np.float32).reshape(6, 128).T)
    bv_b = np.ascontiguousarray(
        np.broadcast_to(np.asarray(bv, np.float32), (128, HID)))
    pk = np.abs(np.asarray(phi_k, np.float32).reshape(H, DH, 1))
    pkv = np.asarray(phi_kv, np.float32).reshape(H, DH, DH)
    aug = np.concatenate([pkv, pk], axis=-1)          # [H, DH, 65]
    aug = np.transpose(aug, (1, 0, 2))                # [DH, H, 65]
    aug = np.ascontiguousarray(np.concatenate([aug, aug], axis=0))  # [128,H,65]
    in_maps = []
    for c in range(NCORES):
        in_maps.append({
            "hidden": np.ascontiguousarray(
                hidden[c * BLOC:(c + 1) * BLOC].reshape(SLOC, HID)),
            "wq": wq, "wk": wk, "wv": wv,
            "bq_p": bq_p, "bk_p": bk_p, "bv_b": bv_b,
            "phikv_aug": aug,
        })
    return in_maps


def kernel(hidden_states, Wq, bq, Wk, bk, Wv, bv, phi_k, phi_kv):
    if "nc" not in _CACHE:
        _CACHE["nc"] = build_nc()
    nc = _CACHE["nc"]
    in_maps = _prep_in_maps(hidden_states, Wq, bq, Wk, bk, Wv, bv,
                            phi_k, phi_kv)
    res = run_bass_kernel_spmd(nc, in_maps, list(range(NCORES)), trace=False)
    out = np.concatenate([res.results[c]["out"] for c in range(NCORES)], axis=0)
    return np.ascontiguousarray(out.reshape(B, S, HID).astype(np.float32))


# revision 8
# speedup vs baseline: 341.2668x; 341.2668x over previous
"""NTK-ViT self-attention (softmax attention + linear-attention correction)
for Trainium2, data-parallel over batch across 8 NeuronCores.

Math (per batch b, head h):
    q = hidden @ Wq.T + bq ; k = .. ; v = ..           (per-head slices of 768)
    A'  = exp(q k^T / sqrt(d))          (no max-subtract: values are O(exp(6)),
                                         and the reference's max-subtract cancels
                                         exactly in the final ratio)
    phi_q = elu(q / d^0.25) + 1 = exp(min(x,0)) + relu(x),  x = q / d^0.25
    ctx = (A' v + phi_q @ phi_kv) / (rowsum(A') + phi_q @ |phi_k|)

On-chip strategy (per core: 2 batches, 12 heads = 6 head-pairs):
  - hidden^T / W^T via PE transposes (fp32); W^T staged in DRAM scratch and
    streamed back per batch (SBUF budget).
  - Projections in fp32r (full-speed, ~15x more precise than bf16 on HW).
  - Q^T, K^T in [o=768 (6 x 128-part tiles = head PAIRS), s=1024] fp32.
  - scores computed transposed: S^T[t, q]; the two heads of a pair run
    CONCURRENTLY on the PE array (K=64 row tiling via base_partition 0/64),
    in fp32r.
  - exp on ACT (scale=1/8 fused), writing bf16 A'^T tiles [t_tile, q].
  - A'v: out[q_tile, 65] psum accumulation with lhsT = A'^T tiles (bf16) and
    rhs = V_aug = [V | ones] (bf16); the ones column yields rowsum(A') free.
    The phi-correction matmul accumulates into the same psum.
  - epilogue: DVE reciprocal of col 64, scale cols 0..63, DMA to DRAM.
"""

import contextlib

import numpy as np

import concourse.bass as bass
import concourse.mybir as mybir
import concourse.tile as tile
import bass_rust
from concourse.masks import make_identity
from concourse.bass_utils import run_bass_kernel_spmd

F32 = mybir.dt.float32
F32R = mybir.dt.float32r
BF16 = mybir.dt.bfloat16
AF = mybir.ActivationFunctionType
ALU = mybir.AluOpType

B, S, HID = 16, 1024, 768
H, DH = 12, 64
NCORES = 8
BLOC = B // NCORES          # batches per core
SLOC = BLOC * S             # 2048 rows of hidden per core
HP = H // 2                 # head pairs
INV_SQRT_D = 1.0 / np.sqrt(DH)          # 0.125
INV_QD = 1.0 / float(DH) ** 0.25        # 1/2.8284


def _split_multi_waits(nc):
    """This walrus build rejects instructions carrying more than one sync
    wait. Hoist extra waits onto standalone EventSemaphore nops emitted
    immediately before the instruction on the same engine (identical
    blocking semantics: the engine stalls on each wait in turn)."""
    ctr = 0
    for fn in nc.m.functions:
        for bb in fn.blocks:
            out = []
            changed = False
            for inst in bb.instructions:
                si = inst.sync_info
                if si is not None and len(si.on_wait) > 1:
                    waits = list(si.on_wait)
                    for w in waits[:-1]:
                        ctr += 1
                        nop = mybir.InstEventSemaphore(
                            name=f"I-waitsplit-{ctr}",
                            engine=inst.engine,
                            ins=[], outs=[],
                            sync_info=bass_rust.SyncInfo(
                                on_wait=[w], on_update=[]),
                        )
                        out.append(nop)
                    inst.sync_info = bass_rust.SyncInfo(
                        on_wait=[waits[-1]], on_update=list(si.on_update))
                    changed = True
                out.append(inst)
            if changed:
                bb.instructions = out


def build_nc():
    nc = bass.Bass()
    hid = nc.declare_dram_parameter("hidden", [SLOC, HID], F32R, isOutput=False)
    w_in = {
        w: nc.declare_dram_parameter(w, [HID, HID], F32R, isOutput=False)
        for w in ("wq", "wk", "wv")
    }
    bq_p = nc.declare_dram_parameter("bq_p", [128, 6], F32, isOutput=False)
    bk_p = nc.declare_dram_parameter("bk_p", [128, 6], F32, isOutput=False)
    bv_b = nc.declare_dram_parameter("bv_b", [128, HID], F32, isOutput=False)
    # [128 (=2x d), H, DH+1]: [phi_kv | |phi_k|], replicated on both 64-halves
    pkv = nc.declare_dram_parameter("phikv_aug", [128, H, DH + 1], F32,
                                    isOutput=False)
    outp = nc.declare_dram_parameter("out", [SLOC, HID], F32, isOutput=True)
    # DRAM scratch for transposed weights [i, o]
    wt_dram = {w: nc.dram_tensor(f"wt_{w}", [HID, HID], F32R)
               for w in ("wq", "wk", "wv")}

    with tile.TileContext(nc) as tc:
        with contextlib.ExitStack() as ctx:
            cpool = ctx.enter_context(tc.tile_pool(name="const", bufs=1))
            big = ctx.enter_context(tc.tile_pool(name="big", bufs=1))
            att = ctx.enter_context(tc.tile_pool(name="att", bufs=3))
            tmp = ctx.enter_context(tc.tile_pool(name="tmp", bufs=2))
            eps_p = ctx.enter_context(tc.tile_pool(name="eps", bufs=6))
            ptr = ctx.enter_context(tc.tile_pool(name="ptr", bufs=1, space="PSUM"))
            pproj = ctx.enter_context(tc.tile_pool(name="pproj", bufs=1, space="PSUM"))
            pqk = ctx.enter_context(tc.tile_pool(name="pqk", bufs=2, space="PSUM"))
            pav = ctx.enter_context(tc.tile_pool(name="pav", bufs=2, space="PSUM"))

            # ---- constants ----
            ident_f = cpool.tile([128, 128], F32)
            make_identity(nc, ident_f[:])
            ident = cpool.tile([128, 128], F32R)
            nc.vector.tensor_copy(out=ident[:], in_=ident_f[:])

            pkv_f = cpool.tile([128, H, DH + 1], F32)
            nc.sync.dma_start(out=pkv_f[:], in_=pkv[:])
            pkv_b = cpool.tile([128, H, DH + 1], BF16)
            nc.vector.tensor_copy(out=pkv_b[:], in_=pkv_f[:])

            bqt = cpool.tile([128, 6], F32)
            nc.sync.dma_start(out=bqt[:], in_=bq_p[:])
            bkt = cpool.tile([128, 6], F32)
            nc.sync.dma_start(out=bkt[:], in_=bk_p[:])
            bvt = cpool.tile([128, HID], F32)
            nc.sync.dma_start(out=bvt[:], in_=bv_b[:])

            # ---- W^T via PE transposes, staged to DRAM scratch ----
            for w in ("wq", "wk", "wv"):
                for ot in range(6):
                    row = tmp.tile([128, HID], F32R, tag="row_f")
                    nc.sync.dma_start(out=row[:],
                                      in_=w_in[w][ot * 128:(ot + 1) * 128, :])
                    for it in range(6):
                        pt = ptr.tile([128, 128], F32R, tag="tr")
                        nc.tensor.transpose(pt[:], row[:, it * 128:(it + 1) * 128],
                                            ident[:])
                        stg = tmp.tile([128, 128], F32R, tag="wtstg")
                        nc.vector.tensor_copy(out=stg[:], in_=pt[:])
                        nc.sync.dma_start(
                            out=wt_dram[w][it * 128:(it + 1) * 128,
                                           ot * 128:(ot + 1) * 128],
                            in_=stg[:])

            for b in range(BLOC):
                # ---- hidden^T: hT[i_part, i_tile, s] (fp32) ----
                hT = big.tile([128, 6, S], F32R, tag="hT")
                for st in range(8):
                    row = tmp.tile([128, HID], F32R, tag="row_f")
                    nc.sync.dma_start(
                        out=row[:],
                        in_=hid[b * S + st * 128: b * S + (st + 1) * 128, :])
                    for it in range(6):
                        pt = ptr.tile([128, 128], F32R, tag="tr")
                        nc.tensor.transpose(pt[:], row[:, it * 128:(it + 1) * 128],
                                            ident[:])
                        nc.vector.tensor_copy(
                            out=hT[:, it, st * 128:(st + 1) * 128], in_=pt[:])

                # ---- Q^T, K^T projections (fp32r): QT[o_part, hp, s] fp32 ----
                QT = big.tile([128, HP, S], F32R, tag="QT")
                KT = big.tile([128, HP, S], F32R, tag="KT")
                for dst, wkey, bias in ((QT, "wq", bqt), (KT, "wk", bkt)):
                    for hp in range(HP):
                        wch = tmp.tile([128, 6, 128], F32R, tag="wch")
                        nc.sync.dma_start(
                            out=wch[:],
                            in_=wt_dram[wkey][:, hp * 128:(hp + 1) * 128]
                                .rearrange("(a p) o -> p a o", p=128))
                        for qc in range(2):
                            ps = pproj.tile([128, 512], F32, tag="proj")
                            for it in range(6):
                                nc.tensor.matmul(
                                    ps[:],
                                    lhsT=wch[:, it, :],
                                    rhs=hT[:, it, qc * 512:(qc + 1) * 512],
                                    start=(it == 0), stop=(it == 5))
                            nc.vector.tensor_scalar(
                                out=dst[:, hp, qc * 512:(qc + 1) * 512],
                                in0=ps[:], scalar1=bias[:, hp:hp + 1],
                                scalar2=None, op0=ALU.add)

                # ---- V (natural [t, o]) + ones column (bf16) ----
                wvT = big.tile([128, 6, HID], F32R, tag="wvT")
                nc.sync.dma_start(
                    out=wvT[:],
                    in_=wt_dram["wv"].rearrange("(a p) o -> p a o", p=128))
                vaug = big.tile([128, 8, H, DH + 1], BF16, tag="vaug")
                nc.any.memset(vaug[:, :, :, DH:DH + 1], 1.0)
                for st in range(8):
                    for nc2 in range(2):
                        ps = pproj.tile([128, 512], F32, tag="proj")
                        for it in range(6):
                            nc.tensor.matmul(
                                ps[:, :384],
                                lhsT=hT[:, it, st * 128:(st + 1) * 128],
                                rhs=wvT[:, it, nc2 * 384:(nc2 + 1) * 384],
                                start=(it == 0), stop=(it == 5))
                        nc.vector.tensor_tensor(
                            out=vaug[:, st, nc2 * 6:(nc2 + 1) * 6, 0:DH],
                            in0=ps[:, :384].rearrange("p (h d) -> p h d", d=DH),
                            in1=bvt[:, nc2 * 384:(nc2 + 1) * 384].rearrange(
                                "p (h d) -> p h d", d=DH),
                            op=ALU.add)

                # ---- phi_q^T = elu(q/d^.25)+1 = exp(min(x,0)) + relu(x) ----
                phiQ = big.tile([128, HP, S], BF16, tag="phiQ")
                for hp in range(HP):
                    t1 = tmp.tile([128, S], F32, tag="phi_min")
                    nc.vector.tensor_scalar_min(t1[:], QT[:, hp, :], 0.0)
                    t2 = tmp.tile([128, S], BF16, tag="phi_exp")
                    nc.scalar.activation(t2[:], t1[:], AF.Exp, scale=INV_QD)
                    nc.vector.tensor_scalar(
                        out=phiQ[:, hp, :], in0=QT[:, hp, :],
                        scalar1=0.0, scalar2=INV_QD, op0=ALU.max, op1=ALU.mult)
                    nc.vector.tensor_tensor(
                        out=phiQ[:, hp, :], in0=phiQ[:, hp, :], in1=t2[:],
                        op=ALU.add)

                # ---- attention per head pair ----
                for hp in range(HP):
                    ATh = [att.tile([128, 8, S], BF16, tag="AT",
                                    name=f"AT_{b}_{hp}_{h2}")
                           for h2 in range(2)]
                    for t in range(8):
                        for h2 in range(2):
                            lo, hi = h2 * 64, (h2 + 1) * 64
                            ps = pqk.tile([128, 1024], F32, tag="qk")
                            for qc in range(2):
                                nc.tensor.matmul(
                                    ps[:, qc * 512:(qc + 1) * 512],
                                    lhsT=KT[lo:hi, hp, t * 128:(t + 1) * 128],
                                    rhs=QT[lo:hi, hp, qc * 512:(qc + 1) * 512],
                                    start=True, stop=True)
                            nc.scalar.activation(ATh[h2][:, t, :], ps[:], AF.Exp,
                                                 scale=INV_SQRT_D)
                    for h2 in range(2):
                        h_abs = hp * 2 + h2
                        lo, hi = h2 * 64, (h2 + 1) * 64
                        for qt in range(8):
                            po = pav.tile([128, DH + 1], F32, tag="av")
                            for t in range(8):
                                nc.tensor.matmul(
                                    po[:],
                                    lhsT=ATh[h2][:, t, qt * 128:(qt + 1) * 128],
                                    rhs=vaug[:, t, h_abs, :],
                                    start=(t == 0), stop=False,
                                    skip_group_check=True)
                            nc.tensor.matmul(
                                po[:],
                                lhsT=phiQ[lo:hi, hp, qt * 128:(qt + 1) * 128],
                                rhs=pkv_b[lo:hi, h_abs, :],
                                start=False, stop=True, skip_group_check=True)
                            rc = eps_p.tile([128, 1], F32, tag="recip")
                            nc.vector.reciprocal(rc[:], po[:, DH:DH + 1])
                            stg = eps_p.tile([128, DH], F32, tag="stage")
                            nc.vector.tensor_scalar(
                                out=stg[:], in0=po[:, 0:DH], scalar1=rc[:],
                                scalar2=None, op0=ALU.mult)
                            nc.sync.dma_start(
                                out=outp[b * S + qt * 128: b * S + (qt + 1) * 128,
                                         h_abs * DH:(h_abs + 1) * DH],
                                in_=stg[:])
    _split_multi_waits(nc)
    return nc


_CACHE = {}


def _prep_in_maps(hidden_states, Wq, bq, Wk, bk, Wv, bv, phi_k, phi_kv):
    hidden = np.ascontiguousarray(
        np.asarray(hidden_states, np.float32).reshape(B, S, HID))
    wq = np.ascontiguousarray(np.asarray(Wq, np.float32))
    wk = np.ascontiguousarray(np.asarray(Wk, np.float32))
    wv = np.ascontiguousarray(np.asarray(Wv, np.float32))
    bq_p = np.ascontiguousarray(np.asarray(bq, np.float32).reshape(6, 128).T)
    bk_p = np.ascontiguousarray(np.asarray(bk, np.float32).reshape(6, 128).T)
    bv_b = np.ascontiguousarray(
        np.broadcast_to(np.asarray(bv, np.float32), (128, HID)))
    pk = np.abs(np.asarray(phi_k, np.float32).reshape(H, DH, 1))
    pkv = np.asarray(phi_kv, np.float32).reshape(H, DH, DH)
    aug = np.concatenate([pkv, pk], axis=-1)          # [H, DH, 65]
    aug = np.transpose(aug, (1, 0, 2))                # [DH, H, 65]
    aug = np.ascontiguousarray(np.concatenate([aug, aug], axis=0))  # [128,H,65]
    in_maps = []
    for c in range(NCORES):
        in_maps.append({
            "hidden": np.ascontiguousarray(
                hidden[c * BLOC:(c + 1) * BLOC].reshape(SLOC, HID)),
            "wq": wq, "wk": wk, "wv": wv,
            "bq_p": bq_p, "bk_p": bk_p, "bv_b": bv_b,
            "phikv_aug": aug,
        })
    return in_maps


def kernel(hidden_states, Wq, bq, Wk, bk, Wv, bv, phi_k, phi_kv):
    if "nc" not in _CACHE:
        _CACHE["nc"] = build_nc()
    nc = _CACHE["nc"]
    in_maps = _prep_in_maps(hidden_states, Wq, bq, Wk, bk, Wv, bv,
                            phi_k, phi_kv)
    res = run_bass_kernel_spmd(nc, in_maps, list(range(NCORES)), trace=False)
    out = np.concatenate([res.results[c]["out"] for c in range(NCORES)], axis=0)
    return np.ascontiguousarray(out.reshape(B, S, HID).astype(np.float32))
